# revision 25
# baseline (speedup 1.0000x reference)
"""Multi-head attention (B=2, N=2048, C=768, H=12, DH=64) on 8 Trainium2 cores.

Sharding: data-parallel on batch (cores 0-3 -> b=0, cores 4-7 -> b=1),
tensor-parallel on heads within each group (3 heads/core: Wq/Wk/Wv column
slices, Wp row slices).  Each core emits its partial projection output
[N, C]; the host sums the 4 partials per batch and adds bp (cheaper than a
device collective at this size).

Per-core dataflow (feature-major, transpose-free, fp16 operands / fp32 psum):
  - host supplies xT = x[b].T  [C, N] in fp16
  - qT,kT [64, N] per head = W.T @ xT       (W natural layout as lhsT)
  - v     [N, 192] token-major from xT as lhsT, with a ones column per head
  - ST    [kj, qi] = kT.T-slice @ qT        (scores, transposed); two K=64
    matmuls packed on disjoint PE row halves per [128,1024] psum tile
    (heads 0+1 paired; head 2 pairs even/odd kj via partition-duplicated k/q)
  - ET    = exp(ST - 4) one ACT op per [128,1024]  (shift cancels in softmax)
  - yT_aug[65, qi] = [v_h | 1].T @ ET accumulated over kj; row 64 = denominator
  - software pipeline: ST(kj+1) issues before yT(kj) so PE never waits on ACT
  - normalize: reciprocal of denom row, stride-0 DMA broadcast, fused mul-copy
  - out[qi, C] partial = yT (stationary) @ Wp rows: K=128 (heads 0+1) + K=64
"""

import math
import os

import ml_dtypes
import numpy as np

import concourse.bacc as bacc
import concourse.bass as bass
import concourse.mybir as mybir
import concourse.tile as tile
from concourse import bass_utils

B, N, C, H, DH = 2, 2048, 768, 12, 64
NCORES = 8
CPG = 4                  # cores per batch group
HPC = H // CPG           # heads per core = 3
MYC = HPC * DH           # per-core feature width = 192
KC = C // 128            # contraction chunks = 6
NTT = N // 128           # token tiles = 16
QB = 512                 # qi block (psum bank width, fp32)
F32 = mybir.dt.float32
MMDT = mybir.dt.float16  # matmul operand dtype: 1cyc/row, 10-bit mantissa
AF = mybir.ActivationFunctionType
OP = mybir.AluOpType

EXP_SHIFT = -4.0         # exp(s + EXP_SHIFT); cancels between num and denom


def _bcast_parts(ap, nparts):
    """Partition-stride-0 broadcast view of a [1, F] AP (DMA source only)."""
    return bass.AP(tensor=ap.tensor, offset=ap.offset,
                   ap=[[0, nparts]] + [list(d) for d in ap.ap[1:]])


def _emit(nc, tc, pools, aps):
    xT, wq, wk, wv, wp, bq, bk, bv, out = (
        aps["xT"], aps["wq"], aps["wk"], aps["wv"], aps["wp"],
        aps["bq"], aps["bk"], aps["bv"], aps["out"],
    )
    persist = pools["persist"]
    et_pool = pools["et"]
    small = pools["small"]
    ostage = pools["ostage"]

    # ---- persistent SBUF tensors ----
    xT_sb = persist.tile([128, KC * N], MMDT, tag="xT_sb")
    wq_sb = persist.tile([128, KC * MYC], MMDT, tag="wq_sb")
    wk_sb = persist.tile([128, KC * MYC], MMDT, tag="wk_sb")
    wv_sb = persist.tile([128, KC * MYC], MMDT, tag="wv_sb")
    wpA = persist.tile([128, C], MMDT, tag="wpA")
    wpB = persist.tile([64, C], MMDT, tag="wpB")
    bqA = persist.tile([128, 1], F32, tag="bqA")
    bqB = persist.tile([64, 1], F32, tag="bqB")
    bkA = persist.tile([128, 1], F32, tag="bkA")
    bkB = persist.tile([64, 1], F32, tag="bkB")
    bv_row = persist.tile([1, MYC], MMDT, tag="bv_row")
    ones = persist.tile([1, 128], MMDT, tag="ones")
    shift_col = persist.tile([128, 1], F32, tag="shift_col")
    qTA = persist.tile([128, N], MMDT, tag="qTA")
    kTA = persist.tile([128, N], MMDT, tag="kTA")
    # head 2 k/q live duplicated on both partition halves (kj even/odd packing)
    qTB = persist.tile([128, N], MMDT, tag="qTB")
    kTB = persist.tile([128, N], MMDT, tag="kTB")
    v_sb = persist.tile([128, NTT * HPC * 65], MMDT, tag="v_sb")
    yTA = persist.tile([128, N], MMDT, tag="yTA")
    yTB = persist.tile([64, N], MMDT, tag="yTB")

    # ---- input DMAs ----
    for kc in range(KC):
        nc.sync.dma_start(out=xT_sb[:, kc * N:(kc + 1) * N],
                          in_=xT[kc * 128:(kc + 1) * 128, :])
        nc.sync.dma_start(out=wq_sb[:, kc * MYC:(kc + 1) * MYC],
                          in_=wq[kc * 128:(kc + 1) * 128, :])
        nc.sync.dma_start(out=wk_sb[:, kc * MYC:(kc + 1) * MYC],
                          in_=wk[kc * 128:(kc + 1) * 128, :])
        nc.sync.dma_start(out=wv_sb[:, kc * MYC:(kc + 1) * MYC],
                          in_=wv[kc * 128:(kc + 1) * 128, :])
    nc.sync.dma_start(out=wpA, in_=wp[0:128, :])
    nc.sync.dma_start(out=wpB, in_=wp[128:MYC, :])
    nc.sync.dma_start(out=bqA, in_=bq[0:128, :])
    nc.sync.dma_start(out=bqB, in_=bq[128:MYC, :])
    nc.sync.dma_start(out=bkA, in_=bk[0:128, :])
    nc.sync.dma_start(out=bkB, in_=bk[128:MYC, :])
    nc.sync.dma_start(out=bv_row, in_=bv)
    ones_f32 = persist.tile([128, 1], F32, tag="ones_f32")
    ones_row_f32 = persist.tile([1, 128], F32, tag="ones_row_f32")
    nc.vector.memset(ones_f32, 1.0)
    nc.vector.memset(ones_row_f32, 1.0)
    nc.vector.tensor_copy(out=ones, in_=ones_row_f32)
    nc.vector.memset(shift_col, EXP_SHIFT)

    # ---- phases 1+2: q/k/v projections (own PSUM pool, released after) ----
    with tc.tile_pool(name="ps_proj", bufs=2, space="PSUM") as ps_proj:
        for wsb, dstA, dstB, biasA, biasB in (
            (wq_sb, qTA, qTB, bqA, bqB),
            (wk_sb, kTA, kTB, bkA, bkB),
        ):
            for mg in range(2):
                M = 128 if mg == 0 else 64
                dst = dstA if mg == 0 else dstB
                bias = biasA if mg == 0 else biasB
                pss = [ps_proj.tile([M, QB], F32, tag="ps_qk", bufs=5,
                                    name=f"ps_qk{_i}")
                       for _i in range(N // QB)]
                for kc in range(KC):  # kc outer: overlap the xT load
                    for nt in range(N // QB):
                        nc.tensor.matmul(
                            pss[nt],
                            wsb[:, kc * MYC + mg * 128: kc * MYC + mg * 128 + M],
                            xT_sb[:, kc * N + nt * QB: kc * N + nt * QB + QB],
                            start=(kc == 0), stop=(kc == KC - 1),
                        )
                for nt in range(N // QB):
                    nc.vector.tensor_scalar(
                        out=dst[0:M, nt * QB:(nt + 1) * QB], in0=pss[nt],
                        scalar1=bias[0:M, :], scalar2=None, op0=OP.add,
                    )
        # duplicate head-2 k/q onto partitions 64..127 (cross-partition: DMA)
        nc.sync.dma_start(out=qTB[64:128, :], in_=qTB[0:64, :])
        nc.sync.dma_start(out=kTB[64:128, :], in_=kTB[0:64, :])

        for nt in range(NTT):
            ps = ps_proj.tile([128, MYC], F32, tag="ps_v")
            for kc in range(KC):
                nc.tensor.matmul(
                    ps,
                    xT_sb[:, kc * N + nt * 128: kc * N + nt * 128 + 128],
                    wv_sb[:, kc * MYC:(kc + 1) * MYC],
                    start=(kc == 0), stop=False,
                )
            nc.tensor.matmul(ps, ones[0:1, 0:128], bv_row,
                             start=False, stop=True)
            for h in range(HPC):
                base = (nt * HPC + h) * 65
                nc.vector.tensor_copy(out=v_sb[:, base:base + 64],
                                      in_=ps[:, h * 64:(h + 1) * 64])
                nc.vector.tensor_copy(out=v_sb[:, base + 64:base + 65],
                                      in_=ones_f32)

    phases = os.environ.get("K_PHASES", "1234")
    if "3" not in phases:
        for i, src_t in enumerate((qTA, kTA, qTB, v_sb)):
            dump = ostage.tile([128, C], F32, name=f"dump{i}")
            nc.vector.tensor_copy(out=dump, in_=src_t[:, 0:C])
            nc.sync.dma_start(out=out[i * 128:(i + 1) * 128, :], in_=dump)
        return

    # ---- phase 3: attention; unit = (head-pair, qi block of 512) ----
    def vh_ap(kj, h):
        base = (kj * HPC + h) * 65
        return v_sb[:, base:base + 65]

    dram_bc = pools["dram_bc"]

    def normalize(yt, ydst, q0):
        rec = small.tile([1, QB], F32, tag="rec")
        nc.vector.reciprocal(rec, yt[64:65, :])
        dr = dram_bc.tile([1, QB], F32)
        nc.sync.dma_start(out=dr, in_=rec)
        bc = small.tile([64, QB], F32, tag="bc_sb")
        nc.sync.dma_start(out=bc, in_=_bcast_parts(dr, 64))
        nc.vector.scalar_tensor_tensor(
            out=ydst[:, q0:q0 + QB], in0=yt[0:64, :], scalar=1.0, in1=bc,
            op0=OP.mult, op1=OP.mult,
        )

    def proj_block(ps_st, qq):
        # projection for qi tiles of block qq; psum carved from st-pool slots
        # (two [128,384] outs in the two banks of one [128,1024] slot)
        for qt in range(qq * 4, qq * 4 + 4):
            stt = ps_st.tile([128, 1024], F32, tag="st", name=f"pj{qt}")
            ob = ostage.tile([128, C], F32, name=f"ob{qt}")
            for nb in range(2):
                po = stt[:, nb * QB: nb * QB + 384]
                nc.tensor.matmul(po, yTA[:, qt * 128:(qt + 1) * 128],
                                 wpA[:, nb * 384:(nb + 1) * 384],
                                 start=True, stop=False)
                nc.tensor.matmul(po, yTB[0:64, qt * 128:(qt + 1) * 128],
                                 wpB[0:64, nb * 384:(nb + 1) * 384],
                                 start=False, stop=True)
                nc.vector.tensor_copy(out=ob[:, nb * 384:(nb + 1) * 384],
                                      in_=po)
            nc.sync.dma_start(out=out[qt * 128:(qt + 1) * 128, :], in_=ob)

    with tc.tile_pool(name="ps_st", bufs=2, space="PSUM") as ps_st, \
         tc.tile_pool(name="ps_yt", bufs=4, space="PSUM") as ps_yt:
        for qq in range(4):
            q0 = qq * QB

            # --- head 2, even/odd kj pairs on the PE array halves ---
            yt2 = ps_yt.tile([65, QB], F32, tag="yt")
            prev = None
            for kp in range(NTT // 2):
                kj0, kj1 = 2 * kp, 2 * kp + 1
                st = ps_st.tile([128, 1024], F32, tag="st")
                nc.tensor.matmul(st[:, 0:QB],
                                 kTB[0:64, kj0 * 128:(kj0 + 1) * 128],
                                 qTB[0:64, q0:q0 + QB], start=True, stop=True)
                nc.tensor.matmul(st[:, QB:1024],
                                 kTB[64:128, kj1 * 128:(kj1 + 1) * 128],
                                 qTB[64:128, q0:q0 + QB], start=True, stop=True)
                et = et_pool.tile([128, 1024], MMDT)
                nc.scalar.activation(et, st, AF.Exp, bias=shift_col[:, :])
                if prev is not None:
                    pet, pkp = prev
                    nc.tensor.matmul(yt2, vh_ap(2 * pkp, 2), pet[:, 0:QB],
                                     start=(pkp == 0), stop=False)
                    nc.tensor.matmul(yt2, vh_ap(2 * pkp + 1, 2),
                                     pet[:, QB:1024], start=False, stop=False)
                prev = (et, kp)
            pet, pkp = prev
            nc.tensor.matmul(yt2, vh_ap(2 * pkp, 2), pet[:, 0:QB],
                             start=(pkp == 0), stop=False)
            nc.tensor.matmul(yt2, vh_ap(2 * pkp + 1, 2), pet[:, QB:1024],
                             start=False, stop=True)
            normalize(yt2, yTB[0:64, :], q0)

            # --- heads 0+1, row-paired on the PE array ---
            yt0 = ps_yt.tile([65, QB], F32, tag="yt")
            yt1 = ps_yt.tile([65, QB], F32, tag="yt")
            prev = None
            for kj in range(NTT):
                st = ps_st.tile([128, 1024], F32, tag="st")
                nc.tensor.matmul(st[:, 0:QB],
                                 kTA[0:64, kj * 128:(kj + 1) * 128],
                                 qTA[0:64, q0:q0 + QB], start=True, stop=True)
                nc.tensor.matmul(st[:, QB:1024],
                                 kTA[64:128, kj * 128:(kj + 1) * 128],
                                 qTA[64:128, q0:q0 + QB], start=True, stop=True)
                et = et_pool.tile([128, 1024], MMDT)
                nc.scalar.activation(et, st, AF.Exp, bias=shift_col[:, :])
                if prev is not None:
                    pet, pkj = prev
                    nc.tensor.matmul(yt0, vh_ap(pkj, 0), pet[:, 0:QB],
                                     start=(pkj == 0), stop=False)
                    nc.tensor.matmul(yt1, vh_ap(pkj, 1), pet[:, QB:1024],
                                     start=(pkj == 0), stop=False)
                prev = (et, kj)
            pet, pkj = prev
            nc.tensor.matmul(yt0, vh_ap(pkj, 0), pet[:, 0:QB],
                             start=False, stop=True)
            nc.tensor.matmul(yt1, vh_ap(pkj, 1), pet[:, QB:1024],
                             start=False, stop=True)
            normalize(yt0, yTA[0:64, :], q0)
            normalize(yt1, yTA[64:128, :], q0)

            # projection for the PREVIOUS block overlaps this block's drain
            if os.environ.get("K_PROJ", "fused") == "fused":
                if qq > 0:
                    proj_block(ps_st, qq - 1)
        if os.environ.get("K_PROJ", "fused") == "fused":
            proj_block(ps_st, 3)
        else:
            for qq in range(4):
                proj_block(ps_st, qq)




def _build_program():
    nc = bacc.Bacc("TRN2", target_bir_lowering=False, debug=False,
                   num_devices=NCORES)
    aps = {
        "xT": nc.dram_tensor("xT", [C, N], MMDT, kind="ExternalInput").ap(),
        "wq": nc.dram_tensor("wq", [C, MYC], MMDT, kind="ExternalInput").ap(),
        "wk": nc.dram_tensor("wk", [C, MYC], MMDT, kind="ExternalInput").ap(),
        "wv": nc.dram_tensor("wv", [C, MYC], MMDT, kind="ExternalInput").ap(),
        "wp": nc.dram_tensor("wp", [MYC, C], MMDT, kind="ExternalInput").ap(),
        "bq": nc.dram_tensor("bq", [MYC, 1], F32, kind="ExternalInput").ap(),
        "bk": nc.dram_tensor("bk", [MYC, 1], F32, kind="ExternalInput").ap(),
        "bv": nc.dram_tensor("bv", [1, MYC], MMDT, kind="ExternalInput").ap(),
        "out": nc.dram_tensor("out", [N, C], F32, kind="ExternalOutput").ap(),
    }
    with tile.TileContext(nc) as tc:
        import contextlib
        with contextlib.ExitStack() as ctx:
            pools = {
                "persist": ctx.enter_context(tc.tile_pool(name="persist", bufs=1)),
                "et": ctx.enter_context(tc.tile_pool(name="et", bufs=3)),
                "small": ctx.enter_context(tc.tile_pool(name="small", bufs=2)),
                "ostage": ctx.enter_context(tc.tile_pool(name="ostage", bufs=2)),
                "dram_bc": ctx.enter_context(
                    tc.tile_pool(name="dram_bc", bufs=2, space="DRAM")),
            }
            _emit(nc, tc, pools, aps)
    nc.compile()
    return nc


_PROGRAM_CACHE = {}


def _get_program():
    if "nc" not in _PROGRAM_CACHE:
        _PROGRAM_CACHE["nc"] = _build_program()
    return _PROGRAM_CACHE["nc"]


def make_in_maps(x, Wq, bq, Wk, bk, Wv, bv, Wp, bp):
    scale = 1.0 / math.sqrt(DH)
    xTb = [np.ascontiguousarray(x[b].T) for b in range(B)]
    wire = mybir.dt.np(MMDT)
    in_maps = []
    for c in range(NCORES):
        b, hg = c // CPG, c % CPG
        cols = slice(hg * MYC, (hg + 1) * MYC)
        in_maps.append({
            "xT": xTb[b].astype(wire),
            "wq": (np.ascontiguousarray(Wq[:, cols]) * np.float32(scale)).astype(wire),
            "wk": np.ascontiguousarray(Wk[:, cols]).astype(wire),
            "wv": np.ascontiguousarray(Wv[:, cols]).astype(wire),
            "wp": np.ascontiguousarray(Wp[cols, :]).astype(wire),
            "bq": (bq[cols] * np.float32(scale)).reshape(MYC, 1).copy(),
            "bk": bk[cols].reshape(MYC, 1).copy(),
            "bv": bv[cols].reshape(1, MYC).astype(wire),
        })
    return in_maps


def assemble(results, bp):
    out = np.empty((B, N, C), np.float32)
    for b in range(B):
        acc = results[b * CPG]["out"].astype(np.float64)
        for c in range(b * CPG + 1, (b + 1) * CPG):
            acc = acc + results[c]["out"]
        out[b] = (acc + bp.astype(np.float64)).astype(np.float32)
    return out


def kernel(x, Wq, bq, Wk, bk, Wv, bv, Wp, bp, **extra_kwargs):
    x = np.asarray(x, np.float32)
    Wq = np.asarray(Wq, np.float32)
    Wk = np.asarray(Wk, np.float32)
    Wv = np.asarray(Wv, np.float32)
    Wp = np.asarray(Wp, np.float32)
    bq = np.asarray(bq, np.float32)
    bk = np.asarray(bk, np.float32)
    bv = np.asarray(bv, np.float32)
    bp = np.asarray(bp, np.float32)

    nc = _get_program()
    in_maps = make_in_maps(x, Wq, bq, Wk, bk, Wv, bv, Wp, bp)
    res = bass_utils.run_bass_kernel_spmd(nc, in_maps,
                                          core_ids=list(range(NCORES)))
    return assemble(res.results, bp)
